# revision 3
# baseline (speedup 1.0000x reference)
"""MoE FFN (E=8 experts, top-2) — expert-parallel Bass/Tile kernel for 8 TRN2 cores.

Strategy:
  - Host computes the (tiny) router: logits = x @ gate_w.T, top-2 per token,
    renormalized weights (= sigmoid of logit differences).  Token n is
    dispatched to cores e1(n), e2(n).
  - All matmul operands are bf16 (PE runs bf16 at the same 1 col/cycle rate as
    float32r, but DMA bytes halve and Fast Weight Load engages, hiding
    LDWEIGHTS).  Accumulation stays fp32 in PSUM; output returns fp32.
  - Capacity C adapts to the actual max expert load (rounded to 16), so no
    fixed-1152 padding compute.  One token block: the whole [H, C] hidden
    fits SBUF in bf16, so w1/w2 stream from HBM exactly once (~23 MB/core).
  - mm1: hT[hc] = gelu(w1.T @ xgT + b1) per 128-row h-chunk, accumulating
    over 8 d-chunks; tokens split into ceil(C/512) column subtiles.
  - mm2 computes the TRANSPOSED output: yT[d, n] = w2[h, d].T @ hT[h, n],
    accumulating over 32 h-chunks, streaming token columns — so the adaptive
    capacity cut applies to both matmuls, and w2 needs no host transpose.
  - Gate weighting + combine happen on host (linear post-op, negligible cost).
  - PE warm-up matmuls on scratch SBUF cover the initial DMA latency and
    release the HAM clock throttle before real work arrives.
"""

import re

import numpy as np
import ml_dtypes

import bass_rust
import concourse.bass as bass
import concourse.mybir as mybir
import concourse.tile as tile
from concourse import bacc, bass_utils

P = 128
D_MODEL = 1024
D_HID = 4096
E = 8
TOP_K = 2
N_CORES = 8

DC = D_MODEL // P          # 8 d-chunks (contraction for mm1)
HC = D_HID // P            # 32 h-chunks
HGW = 1024                 # w1 group tile width (8 h-chunks per group)
NHG = D_HID // HGW         # 4 groups
DCQ = D_MODEL // P         # 8 output d-chunks for mm2

F32 = mybir.dt.float32
BF16 = mybir.dt.bfloat16
NP_BF16 = ml_dtypes.bfloat16

N_WARM = 16                # PE warm-up matmuls (~2.4 us) to cover head DMA


def _subs_for(C):
    """Split C token columns into ceil(C/512) near-equal subtiles (PSUM bank
    limit is 512 fp32 columns), each a multiple of 8, descending so the very
    last chain/evict/store is the smallest."""
    n = -(-C // 512)
    base = C // n
    sizes = []
    rem = C
    for i in range(n - 1):
        s = min(512, ((base + 7) // 8) * 8)
        sizes.append(s)
        rem -= s
    sizes.append(rem)
    sizes.sort(reverse=True)
    assert sum(sizes) == C and all(s <= 512 for s in sizes)
    return sizes


_tail_patched = False


def _patch_light_tail():
    """Replace Tile's end-of-context machinery (multi-wait drain + two
    all-engine EVSEM barriers + semaphore range-clears, ~10us on HW) with
    single-wait drains on the sync engine covering every logical proc's final
    tick.  The NEFF is executed once per load in this flow, so semaphores
    need not be recycled."""
    global _tail_patched
    if _tail_patched:
        return
    _tail_patched = True

    def _drain_and_barrier(self, tick_clock, wait_clock):
        gc = tick_clock.global_clock
        ticks = eval(re.match(r"VectorClock\((.*)\)", repr(gc)).group(1))
        n = len(ticks)
        for i, v in enumerate(ticks):
            if v > 0:
                vc = bass_rust.VectorClock(
                    [v if j == i else 0 for j in range(n)])
                w = self.nc.sync.drain()
                wait_clock.add_sem_waits(
                    w.ins,
                    bass_rust.ScopedClock({None: vc}),
                    bass_rust.ScopedClock({}),
                )
        popped = self.nc._tile_sem_poison_stack.pop()
        assert popped is self._sem_poison

    tile.TileContext._drain_and_barrier = _drain_and_barrier


def build_nc(C):
    _patch_light_tail()
    SUBS = _subs_for(C)
    nc = bacc.Bacc("TRN2", target_bir_lowering=False, debug=False,
                   num_devices=N_CORES)

    # Inputs, pre-tiled on host into consumption order (all contiguous DMAs):
    #   xgt  [DC, P, C]        bf16  xgt[dc, p, n] = Xg[n, dc*128+p]
    #   w1t  [NHG, DC, P, HGW] bf16  w1t[hg, dc, p, j] = w1[dc*128+p, hg*1024+j]
    #   w2t  [HC, P, D]        bf16  w2t[hc, p, j] = w2[hc*128+p, j]
    #   b1t  [P, HC]           f32   b1t[p, hc] = b1[hc*128+p]
    # Output:
    #   ygt  [D, C]            f32   ygt[d, n] = y[n, d]  (gate applied on host)
    xgt = nc.dram_tensor("xgt", [DC, P, C], BF16, kind="ExternalInput")
    w1t = nc.dram_tensor("w1t", [NHG, DC, P, HGW], BF16, kind="ExternalInput")
    w2t = nc.dram_tensor("w2t", [HC, P, D_MODEL], BF16, kind="ExternalInput")
    b1t = nc.dram_tensor("b1t", [P, HC], F32, kind="ExternalInput")
    ygt = nc.dram_tensor("ygt", [D_MODEL, C], F32, kind="ExternalOutput")

    with tile.TileContext(nc) as tc:
        with (
            tc.tile_pool(name="const", bufs=1) as const,
            tc.tile_pool(name="xg", bufs=1) as xg_pool,
            tc.tile_pool(name="w1", bufs=16) as w1_pool,
            tc.tile_pool(name="w2", bufs=1) as w2_pool,
            tc.tile_pool(name="ht", bufs=1) as ht_pool,
            tc.tile_pool(name="yo", bufs=4) as yo_pool,
            tc.tile_pool(name="ps1", bufs=4, space="PSUM") as ps1,
            tc.tile_pool(name="ps2", bufs=4, space="PSUM") as ps2,
        ):
            b1_sb = const.tile([P, HC], F32, name="b1sb")
            nc.sync.dma_start(out=b1_sb[:], in_=b1t[:, :])
            warm = const.tile([P, 512], BF16, name="warm")
            wdump = const.tile([P, 8], F32, name="wdump")
            nc.vector.memset(warm[:], 0.0)
            # preload the ACT gelu table while the head DMAs stream
            nc.scalar.activation(wdump[:, :1], warm[:, :1],
                                 mybir.ActivationFunctionType.Gelu, bias=0.0)
            # PE warm-up: releases the HAM clock throttle (~3.4us window) and
            # keeps PE busy until the first real operands land
            ps_w = ps1.tile([P, SUBS[0]], F32, name="ps1")
            for _ in range(N_WARM):
                nc.tensor.matmul(ps_w[:], lhsT=warm[:, :P],
                                 rhs=warm[:, :SUBS[0]], start=True, stop=True)

            # head DMAs: xg[dc] and w1 group-0 tile dc, paired across the two
            # HWDGE rings so the first mm1 chain can consume them in dc order
            xg_sb = []
            w1_cache = {}
            for dc in range(DC):
                t = xg_pool.tile([P, C], BF16, name=f"xg{dc}")
                eng = nc.sync if dc % 2 == 0 else nc.scalar
                eng.dma_start(out=t[:], in_=xgt[dc, :, :])
                xg_sb.append(t)
                w1_sb = w1_pool.tile([P, HGW], BF16, name="w1sb")
                eng2 = nc.scalar if dc % 2 == 0 else nc.sync
                eng2.dma_start(out=w1_sb[:], in_=w1t[0, dc, :, :])
                w1_cache[(0, dc)] = w1_sb

            w2_sb = [None] * HC

            # ---- mm1: hT[hc] = gelu(w1.T @ xgT + b1), tokens in SUBS cols ----
            ht_tiles = []
            for hc in range(HC):
                hg, k = divmod(hc, HC // NHG)
                # prefetch next w1 group, one tile per hc
                nhg = hg + 1
                if nhg < NHG:
                    w1_sb = w1_pool.tile([P, HGW], BF16, name="w1sb")
                    eng = nc.sync if (nhg + k) % 2 == 0 else nc.scalar
                    eng.dma_start(out=w1_sb[:], in_=w1t[nhg, k, :, :])
                    w1_cache[(nhg, k)] = w1_sb
                # prefetch w2, two tiles per hc through mid-mm1
                if 8 <= hc < 24:
                    for j in range(2):
                        w2i = (hc - 8) * 2 + j
                        t = w2_pool.tile([P, D_MODEL], BF16, name=f"w2sb{w2i}")
                        eng = nc.sync if (w2i % 2) == 0 else nc.scalar
                        eng.dma_start(out=t[:], in_=w2t[w2i, :, :])
                        w2_sb[w2i] = t

                ht = ht_pool.tile([P, C], BF16, name=f"ht{hc}")
                sub0 = 0
                for SUB in SUBS:
                    ps = ps1.tile([P, SUB], F32, name="ps1")
                    for dc in range(DC):
                        nc.tensor.matmul(
                            ps[:],
                            lhsT=w1_cache[(hg, dc)][:, k * P:(k + 1) * P],
                            rhs=xg_sb[dc][:, sub0:sub0 + SUB],
                            start=(dc == 0),
                            stop=(dc == DC - 1),
                        )
                    nc.scalar.activation(
                        ht[:, sub0:sub0 + SUB], ps[:],
                        mybir.ActivationFunctionType.Gelu,
                        bias=b1_sb[:, hc:hc + 1],
                    )
                    sub0 += SUB
                ht_tiles.append(ht)

            # ---- mm2: yT[dq, n] = sum_hc w2[hc, dq].T @ hT[hc, n] ----
            ei = 0
            for dq in range(DCQ):
                sub0 = 0
                for SUB in SUBS:
                    ps = ps2.tile([P, SUB], F32, name="ps2")
                    for hc in range(HC):
                        nc.tensor.matmul(
                            ps[:],
                            lhsT=w2_sb[hc][:, dq * P:(dq + 1) * P],
                            rhs=ht_tiles[hc][:, sub0:sub0 + SUB],
                            start=(hc == 0),
                            stop=(hc == HC - 1),
                        )
                    yo = yo_pool.tile([P, SUB], F32, name="yo")
                    if ei % 2 == 0:
                        nc.vector.tensor_copy(yo[:], ps[:])
                    else:
                        nc.scalar.copy(yo[:], ps[:])
                    eng = nc.sync if ei % 2 == 0 else nc.scalar
                    eng.dma_start(
                        out=ygt[dq * P:(dq + 1) * P, sub0:sub0 + SUB],
                        in_=yo[:],
                    )
                    ei += 1
                    sub0 += SUB
    nc.compile()
    return nc


_NC_CACHE = {}
TRACE = False
LAST_RESULTS = None


def _get_nc(C):
    if C not in _NC_CACHE:
        _NC_CACHE[C] = build_nc(C)
    return _NC_CACHE[C]


def kernel(x, gate_w, w1, b1, w2, b2):
    x = np.asarray(x, dtype=np.float32)
    gate_w = np.asarray(gate_w, dtype=np.float32)
    w1 = np.asarray(w1, dtype=np.float32)
    b1 = np.asarray(b1, dtype=np.float32)
    w2 = np.asarray(w2, dtype=np.float32)
    b2 = np.asarray(b2, dtype=np.float32)

    B, T, D = x.shape
    N = B * T
    xf = x.reshape(N, D)

    # ---- router (host; 0.05% of model FLOPs — this is the sharding step) ----
    logits = xf @ gate_w.T                           # [N, E]
    order = np.argsort(-logits, axis=1, kind="stable")
    i1, i2 = order[:, 0], order[:, 1]
    l1 = logits[np.arange(N), i1].astype(np.float64)
    l2 = logits[np.arange(N), i2].astype(np.float64)
    g1 = (1.0 / (1.0 + np.exp(l2 - l1))).astype(np.float32)
    g2 = (1.0 - g1).astype(np.float32)

    # ---- dispatch: gather per-expert tokens, pre-tile, cast to bf16 ----
    idx_per_e = []
    gv_per_e = []
    cnts = []
    for e in range(E):
        sel1 = np.nonzero(i1 == e)[0]
        sel2 = np.nonzero(i2 == e)[0]
        idx = np.concatenate([sel1, sel2])
        gv = np.concatenate([g1[sel1], g2[sel2]])
        idx_per_e.append(idx)
        gv_per_e.append(gv)
        cnts.append(idx.shape[0])

    C = max(512, ((max(cnts) + 15) // 16) * 16)      # capacity, 16-aligned

    in_maps = []
    for e in range(E):
        idx = idx_per_e[e]
        cnt = cnts[e]
        xg = np.zeros((C, D), np.float32)
        xg[:cnt] = xf[idx]
        xgt = np.ascontiguousarray(xg.T).astype(NP_BF16).reshape(DC, P, C)
        w1t = np.ascontiguousarray(
            w1[e].reshape(DC, P, NHG, HGW).transpose(2, 0, 1, 3)
        ).astype(NP_BF16)
        w2t = w2[e].reshape(HC, P, D_MODEL).astype(NP_BF16)
        b1t = np.ascontiguousarray(b1[e].reshape(HC, P).T)
        in_maps.append({"xgt": xgt, "w1t": w1t, "w2t": w2t, "b1t": b1t})

    nc = _get_nc(C)
    res = bass_utils.run_bass_kernel_spmd(
        nc, in_maps, core_ids=list(range(N_CORES)), trace=TRACE)
    global LAST_RESULTS
    LAST_RESULTS = res

    # ---- combine (host): yT -> gate-weighted scatter-add.  Each token occurs
    # in exactly 2 experts, never twice in one, so fancy-index += is safe ----
    out = np.zeros((N, D), np.float32)
    for e in range(E):
        idx = idx_per_e[e]
        cnt = idx.shape[0]
        ygt = res.results[e]["ygt"]                  # [D, C] f32
        out[idx] += (ygt[:, :cnt] * gv_per_e[e][None, :]).T

    if np.any(b2):
        gate_full = np.zeros((N, E), np.float32)
        gate_full[np.arange(N), i1] = g1
        gate_full[np.arange(N), i2] = g2
        out += gate_full @ b2.reshape(E, D)

    return out.reshape(B, T, D)
